# revision 10
# baseline (speedup 1.0000x reference)
"""Multi-head attention + sigmoid gating kernel for 8 TRN2 NeuronCores.

Problem (dense_transformer):
    C_IN=512, C=64, N_HEAD=8, B=4, S=2048
    q = (x @ Wq.T)/sqrt(C); k = x @ Wk.T; v = x @ Wv.T   (split heads)
    aw = q k^T + bias[h] + mask_offset;  P = softmax(aw)
    attn = P v ; out1 = attn @ Wo.T + bo
    out  = sigmoid(out1 @ Wg.T + bg) * out1

Sharding: one head per core (bias traffic is minimized: each core reads only
bias[h], reused across all 4 batches). Attention math is done in an
"transposed" orientation (features on partitions, tokens on free axis):
  - projections produce qT/kT (64, TOK) and v (token-tile, 64) directly
  - QK^T produces aw.T (sk on partitions, sq on free)
  - softmax: P.T = exp(aw.T + mask_col) * exp(bias).T  (exp(bias) precomputed
    on host, transposed so DMA is contiguous; mask folded into ACT bias as a
    per-partition column of -1e8)
  - PV: lhsT = [v | ones] so the PSUM result is [attnU.T ; sumexp] in one
    accumulation group; normalization applied post-hoc via batched reciprocal
  - out-projection is feature-sharded (each core computes its head's 64 rows
    of out1.T) with an AllGather between, twice (out1, then gating).
Host side: shard/cast inputs (bf16), run SPMD, reassemble (512, TOK) -> (B,S,C_IN).
"""

import numpy as np
import ml_dtypes

import concourse.bass as bass
import concourse.mybir as mybir
import concourse.tile as tile
from concourse import bacc
from concourse import bass_utils

C_IN = 512
C = 64
H = 8
B = 4
S = 2048
TOK = B * S          # 8192
NCORES = 8
KT = C_IN // 128     # 4 contraction k-tiles
NCH = TOK // 512     # 16 token chunks of 512
NT = TOK // 128      # 64 token tiles of 128
SKT = S // 128       # 16 key tiles per batch
SQC = 4              # query chunks of 512 per batch

BF16 = ml_dtypes.bfloat16
dtb = mybir.dt.bfloat16
dtf = mybir.dt.float32
AF = mybir.ActivationFunctionType


def _body(tc, xT, ebT, moff, wqk, wv, wo, wg, bo, bg, outT):
    nc = tc.nc
    with (
        tc.tile_pool(name="const", bufs=1) as const,
        tc.tile_pool(name="resid", bufs=1) as resid,
        tc.tile_pool(name="xload", bufs=2) as xload,
        tc.tile_pool(name="agload", bufs=2) as agload,
        tc.tile_pool(name="ebload", bufs=2) as ebload,
        tc.tile_pool(name="ptp", bufs=16) as ptp,
        tc.tile_pool(name="pnp", bufs=4) as pnp,
        tc.tile_pool(name="small", bufs=3) as small,
        tc.tile_pool(name="psA", bufs=4, space="PSUM") as psA,
        tc.tile_pool(name="psB", bufs=2, space="PSUM") as psB,
        tc.tile_pool(name="psC", bufs=2, space="PSUM") as psC,
        tc.tile_pool(name="dram", bufs=1, space="DRAM") as dram,
    ):
        # ---------------- constants ----------------
        w_qk = const.tile([128, KT, 128], dtb)
        nc.sync.dma_start(out=w_qk, in_=wqk.rearrange("(k p) m -> p k m", p=128))
        w_v = const.tile([128, KT, C], dtb)
        nc.sync.dma_start(out=w_v, in_=wv.rearrange("(k p) m -> p k m", p=128))
        w_o = const.tile([128, KT, C], dtb)
        nc.sync.dma_start(out=w_o, in_=wo.rearrange("(k p) m -> p k m", p=128))
        w_g = const.tile([128, KT, C], dtb)
        nc.sync.dma_start(out=w_g, in_=wg.rearrange("(k p) m -> p k m", p=128))
        m_off = const.tile([128, B * SKT], dtf)
        nc.sync.dma_start(out=m_off, in_=moff)
        b_o = const.tile([C, 1], dtf)
        nc.sync.dma_start(out=b_o, in_=bo)
        b_g = const.tile([C, 1], dtf)
        nc.sync.dma_start(out=b_g, in_=bg)

        # ---------------- residents ----------------
        qT = resid.tile([C, TOK], dtb)       # q.T, head-local
        kT = resid.tile([C, TOK], dtb)
        v65 = resid.tile([128, NT, C + 1], dtb)   # [v | ones] per 128-token tile
        attnU = resid.tile([C, TOK], dtb)    # attn.T (unnormalized, then in-place normalized)
        sexp = resid.tile([B * SQC, 512], dtf)
        recs = resid.tile([B * SQC, 512], dtf)
        out1b = resid.tile([C, TOK], dtb)    # out1.T rows of this head (bf16)

        nc.vector.memset(v65[:, :, C:C + 1], 1.0)

        # ---------------- P1: projections ----------------
        xTr = xT.rearrange("(k p) t -> p k t", p=128)
        for n in range(NCH):
            sl = slice(n * 512, (n + 1) * 512)
            xt = xload.tile([128, KT, 512], dtb, tag="xt")
            nc.sync.dma_start(out=xt, in_=xTr[:, :, sl])
            ps_qk = psA.tile([128, 512], dtf, tag="mm")
            for k in range(KT):
                nc.tensor.matmul(ps_qk, lhsT=w_qk[:, k, :], rhs=xt[:, k, :],
                                 start=(k == 0), stop=(k == KT - 1))
            nc.vector.tensor_copy(qT[:, sl], ps_qk[0:C, :])
            nc.vector.tensor_copy(kT[:, sl], ps_qk[C:128, :])
            for s4 in range(4):
                ps_v = psC.tile([128, C], dtf, tag="vv")
                for k in range(KT):
                    nc.tensor.matmul(ps_v, lhsT=xt[:, k, s4 * 128:(s4 + 1) * 128],
                                     rhs=w_v[:, k, :],
                                     start=(k == 0), stop=(k == KT - 1))
                nc.vector.tensor_copy(v65[:, n * 4 + s4, 0:C], ps_v)

        # ---------------- P2: attention ----------------
        ebTr = ebT.rearrange("(j p) q -> p j q", p=128)
        for sqc in range(SQC):
            ebt = ebload.tile([128, SKT, 512], dtb, tag="ebt")
            nc.sync.dma_start(out=ebt, in_=ebTr[:, :, sqc * 512:(sqc + 1) * 512])
            for b in range(B):
                gq = b * S + sqc * 512
                pts = []
                for j in range(SKT):
                    gk = b * S + j * 128
                    ps_aw = psA.tile([128, 512], dtf, tag="mm")
                    nc.tensor.matmul(ps_aw, lhsT=kT[:, gk:gk + 128],
                                     rhs=qT[:, gq:gq + 512], start=True, stop=True)
                    pn = pnp.tile([128, 512], dtb, tag="pn")
                    nc.scalar.activation(pn, ps_aw, AF.Exp,
                                         bias=m_off[:, b * SKT + j:b * SKT + j + 1],
                                         scale=1.0)
                    pt = ptp.tile([128, 512], dtb, tag="pt")
                    nc.vector.tensor_mul(pt, pn, ebt[:, j, :])
                    pts.append(pt)
                ps_pv = psB.tile([C + 1, 512], dtf, tag="pv")
                for j in range(SKT):
                    nc.tensor.matmul(ps_pv, lhsT=v65[:, b * SKT + j, :], rhs=pts[j],
                                     start=(j == 0), stop=(j == SKT - 1))
                ci = b * SQC + sqc
                nc.vector.tensor_copy(attnU[:, gq:gq + 512], ps_pv[0:C, :])
                # engine writes must start at partition 0/32/64/96: stage the
                # sumexp row at partition 0, then DMA it into row ci.
                sx = small.tile([1, 512], dtf, tag="sx")
                nc.vector.tensor_copy(sx, ps_pv[C:C + 1, :])
                nc.sync.dma_start(out=sexp[ci:ci + 1, :], in_=sx)

        # ---------------- P2b: normalize ----------------
        nc.vector.reciprocal(recs, sexp)
        recd = dram.tile([B * SQC, 512], dtf)
        nc.sync.dma_start(out=recd, in_=recs)
        recd_ap = recd[:]
        for b in range(B):
            for sqc in range(SQC):
                ci = b * SQC + sqc
                gq = b * S + sqc * 512
                recB = small.tile([C, 512], dtf, tag="recB")
                row = recd_ap[ci:ci + 1, :]
                rec_bcast = bass.AP(
                    tensor=row.tensor, offset=row.offset,
                    ap=[[0, C]] + list(row.ap[1:]),
                )
                nc.sync.dma_start(out=recB, in_=rec_bcast)
                nc.vector.tensor_mul(attnU[:, gq:gq + 512], attnU[:, gq:gq + 512], recB)

        # ---------------- P3: AllGather attn ----------------
        attn_d = dram.tile([C, TOK], dtb)
        agA_d = dram.tile([C * NCORES, TOK], dtb)
        nc.sync.dma_start(out=attn_d, in_=attnU)
        nc.gpsimd.collective_compute(
            "AllGather", mybir.AluOpType.bypass,
            replica_groups=[list(range(NCORES))],
            ins=[attn_d.opt()], outs=[agA_d.opt()],
        )

        # ---------------- P4: out1 = attn @ Wo.T + bo (this head's 64 rows) ----
        agAr = agA_d[:].rearrange("(k p) t -> p k t", p=128)
        for n in range(NCH):
            sl = slice(n * 512, (n + 1) * 512)
            at = agload.tile([128, KT, 512], dtb, tag="agt")
            nc.sync.dma_start(out=at, in_=agAr[:, :, sl])
            ps_o1 = psA.tile([C, 512], dtf, tag="mm")
            for k in range(KT):
                nc.tensor.matmul(ps_o1, lhsT=w_o[:, k, :], rhs=at[:, k, :],
                                 start=(k == 0), stop=(k == KT - 1))
            nc.scalar.add(out1b[:, sl], ps_o1, b_o[:, 0:1])

        # ---------------- P5: AllGather out1 ----------------
        o1_d = dram.tile([C, TOK], dtb)
        agB_d = dram.tile([C * NCORES, TOK], dtb)
        nc.sync.dma_start(out=o1_d, in_=out1b)
        nc.gpsimd.collective_compute(
            "AllGather", mybir.AluOpType.bypass,
            replica_groups=[list(range(NCORES))],
            ins=[o1_d.opt()], outs=[agB_d.opt()],
        )

        # ---------------- P6: gating ----------------
        agBr = agB_d[:].rearrange("(k p) t -> p k t", p=128)
        for n in range(NCH):
            sl = slice(n * 512, (n + 1) * 512)
            gt = agload.tile([128, KT, 512], dtb, tag="agt")
            nc.sync.dma_start(out=gt, in_=agBr[:, :, sl])
            ps_o2 = psA.tile([C, 512], dtf, tag="mm")
            for k in range(KT):
                nc.tensor.matmul(ps_o2, lhsT=w_g[:, k, :], rhs=gt[:, k, :],
                                 start=(k == 0), stop=(k == KT - 1))
            sig = small.tile([C, 512], dtf, tag="sig")
            nc.scalar.activation(sig, ps_o2, AF.Sigmoid, bias=b_g[:, 0:1], scale=1.0)
            fin = small.tile([C, 512], dtf, tag="fin")
            nc.vector.tensor_mul(fin, sig, out1b[:, sl])
            nc.sync.dma_start(out=outT[:, sl], in_=fin)


_CACHE = {}


def _build():
    if "nc" in _CACHE:
        return _CACHE["nc"]
    nc = bacc.Bacc("TRN2", target_bir_lowering=False, debug=False, num_devices=NCORES)
    aps = {}
    aps["xT"] = nc.dram_tensor("xT", [C_IN, TOK], dtb, kind="ExternalInput").ap()
    aps["ebT"] = nc.dram_tensor("ebT", [S, S], dtb, kind="ExternalInput").ap()
    aps["moff"] = nc.dram_tensor("moff", [128, B * SKT], dtf, kind="ExternalInput").ap()
    aps["wqk"] = nc.dram_tensor("wqk", [C_IN, 128], dtb, kind="ExternalInput").ap()
    aps["wv"] = nc.dram_tensor("wv", [C_IN, C], dtb, kind="ExternalInput").ap()
    aps["wo"] = nc.dram_tensor("wo", [C_IN, C], dtb, kind="ExternalInput").ap()
    aps["wg"] = nc.dram_tensor("wg", [C_IN, C], dtb, kind="ExternalInput").ap()
    aps["bo"] = nc.dram_tensor("bo", [C, 1], dtf, kind="ExternalInput").ap()
    aps["bg"] = nc.dram_tensor("bg", [C, 1], dtf, kind="ExternalInput").ap()
    aps["outT"] = nc.dram_tensor("outT", [C, TOK], dtf, kind="ExternalOutput").ap()
    with tile.TileContext(nc) as tc:
        _body(tc, **aps)
    nc.compile()
    _CACHE["nc"] = nc
    return nc


def _prep_in_maps(inputs):
    x = np.asarray(inputs["x"], dtype=np.float32)
    bias = np.asarray(inputs["bias"], dtype=np.float32)
    mask = np.asarray(inputs["attention_mask"])
    Wq = np.asarray(inputs["Wq"], dtype=np.float32)
    Wk = np.asarray(inputs["Wk"], dtype=np.float32)
    Wv = np.asarray(inputs["Wv"], dtype=np.float32)
    Wo = np.asarray(inputs["Wo"], dtype=np.float32)
    bo = np.asarray(inputs["bo"], dtype=np.float32)
    Wg = np.asarray(inputs["Wg"], dtype=np.float32)
    bg = np.asarray(inputs["bg"], dtype=np.float32)

    xT = np.ascontiguousarray(x.reshape(TOK, C_IN).T).astype(BF16)
    # mask offset columns: moff[p, b*SKT + j] = -1e8 where mask[b, j*128+p]==0
    moff = np.where(mask.reshape(B, SKT, 128) == 0, np.float32(-1e8), np.float32(0.0))
    moff = np.ascontiguousarray(moff.transpose(2, 0, 1).reshape(128, B * SKT))

    in_maps = []
    for h in range(NCORES):
        hs = slice(h * C, (h + 1) * C)
        ebT = np.ascontiguousarray(np.exp(bias[h]).T).astype(BF16)
        wqk = np.empty((C_IN, 128), dtype=BF16)
        wqk[:, 0:C] = (Wq[hs, :] / np.sqrt(C)).T.astype(BF16)
        wqk[:, C:128] = Wk[hs, :].T.astype(BF16)
        in_maps.append({
            "xT": xT,
            "ebT": ebT,
            "moff": moff,
            "wqk": wqk,
            "wv": np.ascontiguousarray(Wv[hs, :].T).astype(BF16),
            "wo": np.ascontiguousarray(Wo.T[:, hs]).astype(BF16),
            "wg": np.ascontiguousarray(Wg.T[:, hs]).astype(BF16),
            "bo": np.ascontiguousarray(bo[hs, None]),
            "bg": np.ascontiguousarray(bg[hs, None]),
        })
    return in_maps


def _run(inputs, trace=False):
    nc = _build()
    in_maps = _prep_in_maps(inputs)
    res = bass_utils.run_bass_kernel_spmd(
        nc, in_maps, core_ids=list(range(NCORES)), trace=trace,
    )
    full_T = np.concatenate([res.results[h]["outT"] for h in range(NCORES)], axis=0)
    out = np.ascontiguousarray(full_T.T).reshape(B, S, C_IN).astype(np.float32)
    return out, res


def kernel(**inputs):
    out, _ = _run(inputs, trace=False)
    return out


if __name__ == "__main__":
    rng = np.random.default_rng(0)
    fake = {
        "x": rng.standard_normal((B, S, C_IN), dtype=np.float32),
        "bias": rng.standard_normal((H, S, S), dtype=np.float32),
        "attention_mask": (rng.integers(0, 2, (B, S))).astype(np.int32),
        "Wq": (rng.standard_normal((C * H, C_IN), dtype=np.float32) * 0.02),
        "Wk": (rng.standard_normal((C * H, C_IN), dtype=np.float32) * 0.02),
        "Wv": (rng.standard_normal((C * H, C_IN), dtype=np.float32) * 0.02),
        "Wo": (rng.standard_normal((C_IN, C * H), dtype=np.float32) * 0.02),
        "bo": np.zeros((C_IN,), dtype=np.float32),
        "Wg": (rng.standard_normal((C * H, C_IN), dtype=np.float32) * 0.02),
        "bg": np.zeros((C * H,), dtype=np.float32),
    }
    out = kernel(**fake)
    print("kernel ran, out shape", out.shape, "finite:", np.isfinite(out).all())


# revision 19
# speedup vs baseline: 1.3419x; 1.3419x over previous
"""Multi-head attention + sigmoid gating kernel for 8 TRN2 NeuronCores.

Problem (dense_transformer):
    C_IN=512, C=64, N_HEAD=8, B=4, S=2048
    q = (x @ Wq.T)/sqrt(C); k = x @ Wk.T; v = x @ Wv.T   (split heads)
    aw = q k^T + bias[h] + mask_offset;  P = softmax(aw)
    attn = P v ; out1 = attn @ Wo.T + bo
    out  = sigmoid(out1 @ Wg.T + bg) * out1

Sharding: one head per core (bias[h] is the dominant HBM tensor; this gives
each core exactly one 16.8MB slice, reused across all 4 batches).

Orientation: features on partitions, tokens on free axis throughout:
  - projections produce qT/kT (duplicated to 128 partitions for 2-way
    row-packed QK^T matmuls) and v directly
  - QK^T produces aw.T (sk on partitions, sq free), two sk-tiles packed
    per PE pass into the two banks of a (128,1024) PSUM tile
  - P.T = exp(aw.T) * mask01[sk] * exp(bias[h]).T via one ACT pass over the
    (128,1024) pair and one DVE scalar_tensor_tensor per 512 half
  - PV: lhsT = [v | ones] accumulates [attnU.T ; sumexp] in PSUM;
    normalization via per-sq-chunk batched reciprocal + DRAM-broadcast
  - out-projection feature-sharded with AllGathers chunked per sq-block so
    the collectives overlap compute.
P2 is software-pipelined (PV pass lags one chunk) to keep PE dense.
"""

import numpy as np
import ml_dtypes

import concourse.bass as bass
import concourse.mybir as mybir
import concourse.tile as tile
from concourse import bacc
from concourse import bass_utils

C_IN = 512
C = 64
H = 8
B = 4
S = 2048
TOK = B * S          # 8192
NCORES = 8
KT = C_IN // 128     # 4 contraction k-tiles
NCH = TOK // 512     # 16 token chunks of 512
SKT = S // 128       # 16 key tiles per batch
SQC = 4              # query chunks of 512 per batch

BF16 = ml_dtypes.bfloat16
dtb = mybir.dt.bfloat16
dtf = mybir.dt.float32
AF = mybir.ActivationFunctionType
ALU = mybir.AluOpType

PACK_QK = True       # 2-way row-packed QK matmuls


def _body(tc, xT, ebT, m01, wqk, wv, wo, wg, bo, bg, outT):
    nc = tc.nc
    with (
        tc.tile_pool(name="const", bufs=1) as const,
        tc.tile_pool(name="resid", bufs=1) as resid,
        tc.tile_pool(name="xload", bufs=3) as xload,
        tc.tile_pool(name="agload", bufs=2) as agload,
        tc.tile_pool(name="ebload", bufs=3) as ebload,
        tc.tile_pool(name="ptp", bufs=20) as ptp,
        tc.tile_pool(name="pnp", bufs=3) as pnp,
        tc.tile_pool(name="anp", bufs=3) as anp,
        tc.tile_pool(name="o1p", bufs=4) as o1p,
        tc.tile_pool(name="small", bufs=2) as small,
        tc.tile_pool(name="psA", bufs=3, space="PSUM") as psA,
        tc.tile_pool(name="psB", bufs=2, space="PSUM") as psB,
        tc.tile_pool(name="dram", bufs=1, space="DRAM") as dram,
    ):
        # ---------------- constants ----------------
        w_qk = const.tile([128, KT, 128], dtb)
        nc.sync.dma_start(out=w_qk, in_=wqk.rearrange("(k p) m -> p k m", p=128))
        w_v = const.tile([128, KT, C], dtb)
        nc.sync.dma_start(out=w_v, in_=wv.rearrange("(k p) m -> p k m", p=128))
        w_o = const.tile([128, KT, C], dtb)
        nc.sync.dma_start(out=w_o, in_=wo.rearrange("(k p) m -> p k m", p=128))
        w_g = const.tile([128, KT, C], dtb)
        nc.sync.dma_start(out=w_g, in_=wg.rearrange("(k p) m -> p k m", p=128))
        m_01 = const.tile([128, B * SKT], dtf)
        nc.sync.dma_start(out=m_01, in_=m01)
        b_o = const.tile([C, 1], dtf)
        nc.sync.dma_start(out=b_o, in_=bo)
        b_g = const.tile([C, 1], dtf)
        nc.sync.dma_start(out=b_g, in_=bg)

        # ---------------- residents ----------------
        qTd = resid.tile([128, TOK], dtb)    # rows 0:64 and 64:128 both = q.T
        kTd = resid.tile([128, TOK], dtb)
        v65 = resid.tile([128, NCH * 4, C + 1], dtb)   # [v | ones] per 128-token tile
        nc.vector.memset(v65[:, :, C:C + 1], 1.0)

        # ---------------- P1: projections ----------------
        xTr = xT.rearrange("(k p) t -> p k t", p=128)
        for n in range(NCH):
            sl = slice(n * 512, (n + 1) * 512)
            xt = xload.tile([128, KT, 512], dtb, tag="xt")
            nc.sync.dma_start(out=xt, in_=xTr[:, :, sl])
            ps_qk = psA.tile([128, 512], dtf, tag="mm")
            for k in range(KT):
                nc.tensor.matmul(ps_qk, lhsT=w_qk[:, k, :], rhs=xt[:, k, :],
                                 start=(k == 0), stop=(k == KT - 1))
            nc.vector.tensor_copy(qTd[0:C, sl], ps_qk[0:C, :])
            nc.vector.tensor_copy(qTd[C:128, sl], ps_qk[0:C, :])
            nc.vector.tensor_copy(kTd[0:C, sl], ps_qk[C:128, :])
            nc.vector.tensor_copy(kTd[C:128, sl], ps_qk[C:128, :])
            for s4 in range(4):
                ps_v = psB.tile([128, C], dtf, tag="pv")
                for k in range(KT):
                    nc.tensor.matmul(ps_v, lhsT=xt[:, k, s4 * 128:(s4 + 1) * 128],
                                     rhs=w_v[:, k, :],
                                     start=(k == 0), stop=(k == KT - 1))
                nc.vector.tensor_copy(v65[:, n * 4 + s4, 0:C], ps_v)

        # ---------------- P2: attention (software pipelined) ----------------
        ebTr = ebT.rearrange("(j p) q -> p j q", p=128)
        attn_d = []          # per-sqc DRAM staging (64, B*512)
        agA_d = []
        for sqc in range(SQC):
            attn_d.append(dram.tile([C, B * 512], dtb, name=f"attn_d{sqc}"))
            agA_d.append(dram.tile([C * NCORES, B * 512], dtb, name=f"agA_d{sqc}", addr_space="Shared"))
        recd = dram.tile([SQC * B, 512], dtf)

        pend = None          # (sqc, b, pts) whose PV pass is not yet emitted
        attn_t = {}          # sqc -> sbuf tile (C, B*512)
        sexp_t = {}          # sqc -> sbuf tile (B, 512)

        def emit_pv(sqc, b, pts):
            ps_pv = psB.tile([C + 1, 512], dtf, tag="pv")
            for j in range(SKT):
                nc.tensor.matmul(ps_pv, lhsT=v65[:, b * SKT + j, :], rhs=pts[j],
                                 start=(j == 0), stop=(j == SKT - 1))
            nc.vector.tensor_copy(attn_t[sqc][:, b * 512:(b + 1) * 512], ps_pv[0:C, :])
            sx = small.tile([1, 512], dtf, tag="sx")
            nc.vector.tensor_copy(sx, ps_pv[C:C + 1, :])
            nc.sync.dma_start(out=sexp_t[sqc][b:b + 1, :], in_=sx)

        def emit_norm_and_ag(sqc):
            rec = small.tile([B, 512], dtf, tag="rec")
            nc.vector.reciprocal(rec, sexp_t[sqc])
            nc.sync.dma_start(out=recd[sqc * B:(sqc + 1) * B, :], in_=rec)
            for b in range(B):
                recB = small.tile([C, 512], dtf, tag="recB")
                row = recd[:][sqc * B + b:sqc * B + b + 1, :]
                nc.sync.dma_start(out=recB, in_=bass.AP(
                    tensor=row.tensor, offset=row.offset,
                    ap=[[0, C]] + list(row.ap[1:])))
                at = attn_t[sqc][:, b * 512:(b + 1) * 512]
                nc.vector.tensor_mul(at, at, recB)
            nc.sync.dma_start(out=attn_d[sqc], in_=attn_t[sqc])
            nc.gpsimd.collective_compute(
                "AllGather", ALU.bypass,
                replica_groups=[list(range(NCORES))],
                ins=[attn_d[sqc].opt()], outs=[agA_d[sqc].opt()],
            )

        for sqc in range(SQC):
            ebt_lo = ebload.tile([128, SKT // 2, 512], dtb, tag="ebt")
            nc.sync.dma_start(out=ebt_lo, in_=ebTr[:, 0:SKT // 2, sqc * 512:(sqc + 1) * 512])
            ebt_hi = ebload.tile([128, SKT // 2, 512], dtb, tag="ebt")
            nc.sync.dma_start(out=ebt_hi, in_=ebTr[:, SKT // 2:SKT, sqc * 512:(sqc + 1) * 512])
            ebt_halves = (ebt_lo, ebt_hi)
            attn_t[sqc] = anp.tile([C, B * 512], dtb, tag="attn", name=f"attn_t{sqc}")
            sexp_t[sqc] = anp.tile([B, 512], dtf, tag="sexp", name=f"sexp_t{sqc}")
            for b in range(B):
                if pend is not None:
                    emit_pv(*pend)
                    if pend[1] == B - 1:
                        emit_norm_and_ag(pend[0])
                    pend = None
                gq = b * S + sqc * 512
                pts = []
                for jp in range(SKT // 2):
                    jA, jB = 2 * jp, 2 * jp + 1
                    ps_aw = psA.tile([128, 1024], dtf, tag="mm")
                    if PACK_QK:
                        nc.tensor.matmul(ps_aw[:, 0:512],
                                         lhsT=kTd[0:C, b * S + jA * 128:b * S + jA * 128 + 128],
                                         rhs=qTd[0:C, gq:gq + 512],
                                         start=True, stop=True)
                        nc.tensor.matmul(ps_aw[:, 512:1024],
                                         lhsT=kTd[C:128, b * S + jB * 128:b * S + jB * 128 + 128],
                                         rhs=qTd[C:128, gq:gq + 512],
                                         start=True, stop=True)
                    else:
                        nc.tensor.matmul(ps_aw[:, 0:512],
                                         lhsT=kTd[0:C, b * S + jA * 128:b * S + jA * 128 + 128],
                                         rhs=qTd[0:C, gq:gq + 512],
                                         start=True, stop=True)
                        nc.tensor.matmul(ps_aw[:, 512:1024],
                                         lhsT=kTd[0:C, b * S + jB * 128:b * S + jB * 128 + 128],
                                         rhs=qTd[0:C, gq:gq + 512],
                                         start=True, stop=True)
                    pn = pnp.tile([128, 1024], dtb, tag="pn")
                    nc.scalar.activation(pn, ps_aw, AF.Exp)
                    for half, j in ((0, jA), (1, jB)):
                        pt = ptp.tile([128, 512], dtb, tag="pt")
                        nc.vector.scalar_tensor_tensor(
                            pt, pn[:, half * 512:(half + 1) * 512],
                            m_01[:, b * SKT + j:b * SKT + j + 1],
                            ebt_halves[j // (SKT // 2)][:, j % (SKT // 2), :],
                            op0=ALU.mult, op1=ALU.mult,
                        )
                        pts.append(pt)
                pend = (sqc, b, pts)
        emit_pv(*pend)
        emit_norm_and_ag(pend[0])

        # ---------------- P4: out1 = attn @ Wo.T + bo (this head's rows) -----
        out1_t = {}
        agB_d = []
        o1_d = []
        for sqc in range(SQC):
            o1_d.append(dram.tile([C, B * 512], dtb, name=f"o1_d{sqc}"))
            agB_d.append(dram.tile([C * NCORES, B * 512], dtb, name=f"agB_d{sqc}", addr_space="Shared"))
        for sqc in range(SQC):
            agAr = agA_d[sqc][:].rearrange("(k p) t -> p k t", p=128)
            out1_t[sqc] = o1p.tile([C, B * 512], dtb, tag="out1", name=f"out1_t{sqc}")
            for bp in range(B // 2):
                at = agload.tile([128, KT, 1024], dtb, tag="agt")
                nc.sync.dma_start(out=at, in_=agAr[:, :, bp * 1024:(bp + 1) * 1024])
                ps_o1 = psA.tile([C, 1024], dtf, tag="mm")
                for half in range(2):
                    hs = slice(half * 512, (half + 1) * 512)
                    for k in range(KT):
                        nc.tensor.matmul(ps_o1[:, hs], lhsT=w_o[:, k, :],
                                         rhs=at[:, k, hs],
                                         start=(k == 0), stop=(k == KT - 1))
                nc.scalar.add(out1_t[sqc][:, bp * 1024:(bp + 1) * 1024], ps_o1,
                              b_o[:, 0:1])
            nc.sync.dma_start(out=o1_d[sqc], in_=out1_t[sqc])
            nc.gpsimd.collective_compute(
                "AllGather", ALU.bypass,
                replica_groups=[list(range(NCORES))],
                ins=[o1_d[sqc].opt()], outs=[agB_d[sqc].opt()],
            )

        # ---------------- P6: gating ----------------
        for sqc in range(SQC):
            agBr = agB_d[sqc][:].rearrange("(k p) t -> p k t", p=128)
            for bp in range(B // 2):
                gt = agload.tile([128, KT, 1024], dtb, tag="agt")
                nc.sync.dma_start(out=gt, in_=agBr[:, :, bp * 1024:(bp + 1) * 1024])
                ps_o2 = psA.tile([C, 1024], dtf, tag="mm")
                for half in range(2):
                    hs = slice(half * 512, (half + 1) * 512)
                    for k in range(KT):
                        nc.tensor.matmul(ps_o2[:, hs], lhsT=w_g[:, k, :],
                                         rhs=gt[:, k, hs],
                                         start=(k == 0), stop=(k == KT - 1))
                sig = small.tile([C, 1024], dtf, tag="sig")
                nc.scalar.activation(sig, ps_o2, AF.Sigmoid, bias=b_g[:, 0:1],
                                     scale=1.0)
                fin = small.tile([C, 1024], dtf, tag="fin")
                nc.vector.tensor_mul(fin, sig,
                                     out1_t[sqc][:, bp * 1024:(bp + 1) * 1024])
                for half, b in ((0, 2 * bp), (1, 2 * bp + 1)):
                    gq = b * S + sqc * 512
                    nc.sync.dma_start(out=outT[:, gq:gq + 512],
                                      in_=fin[:, half * 512:(half + 1) * 512])


_CACHE = {}


def _build():
    if "nc" in _CACHE:
        return _CACHE["nc"]
    nc = bacc.Bacc("TRN2", target_bir_lowering=False, debug=False, num_devices=NCORES)
    aps = {}
    aps["xT"] = nc.dram_tensor("xT", [C_IN, TOK], dtb, kind="ExternalInput").ap()
    aps["ebT"] = nc.dram_tensor("ebT", [S, S], dtb, kind="ExternalInput").ap()
    aps["m01"] = nc.dram_tensor("m01", [128, B * SKT], dtf, kind="ExternalInput").ap()
    aps["wqk"] = nc.dram_tensor("wqk", [C_IN, 128], dtb, kind="ExternalInput").ap()
    aps["wv"] = nc.dram_tensor("wv", [C_IN, C], dtb, kind="ExternalInput").ap()
    aps["wo"] = nc.dram_tensor("wo", [C_IN, C], dtb, kind="ExternalInput").ap()
    aps["wg"] = nc.dram_tensor("wg", [C_IN, C], dtb, kind="ExternalInput").ap()
    aps["bo"] = nc.dram_tensor("bo", [C, 1], dtf, kind="ExternalInput").ap()
    aps["bg"] = nc.dram_tensor("bg", [C, 1], dtf, kind="ExternalInput").ap()
    aps["outT"] = nc.dram_tensor("outT", [C, TOK], dtf, kind="ExternalOutput").ap()
    with tile.TileContext(nc) as tc:
        _body(tc, **aps)
    nc.compile()
    _CACHE["nc"] = nc
    return nc


def _prep_in_maps(inputs):
    x = np.asarray(inputs["x"], dtype=np.float32)
    bias = np.asarray(inputs["bias"], dtype=np.float32)
    mask = np.asarray(inputs["attention_mask"])
    Wq = np.asarray(inputs["Wq"], dtype=np.float32)
    Wk = np.asarray(inputs["Wk"], dtype=np.float32)
    Wv = np.asarray(inputs["Wv"], dtype=np.float32)
    Wo = np.asarray(inputs["Wo"], dtype=np.float32)
    bo = np.asarray(inputs["bo"], dtype=np.float32)
    Wg = np.asarray(inputs["Wg"], dtype=np.float32)
    bg = np.asarray(inputs["bg"], dtype=np.float32)

    xT = np.ascontiguousarray(x.reshape(TOK, C_IN).T).astype(BF16)
    # mask columns: m01[p, b*SKT + j] = 0.0 where mask[b, j*128+p]==0 else 1.0
    m01 = (mask.reshape(B, SKT, 128) != 0).astype(np.float32)
    m01 = np.ascontiguousarray(m01.transpose(2, 0, 1).reshape(128, B * SKT))

    in_maps = []
    for h in range(NCORES):
        hs = slice(h * C, (h + 1) * C)
        ebT = np.ascontiguousarray(np.exp(bias[h]).T).astype(BF16)
        wqk = np.empty((C_IN, 128), dtype=BF16)
        wqk[:, 0:C] = (Wq[hs, :] / np.sqrt(C)).T.astype(BF16)
        wqk[:, C:128] = Wk[hs, :].T.astype(BF16)
        in_maps.append({
            "xT": xT,
            "ebT": ebT,
            "m01": m01,
            "wqk": wqk,
            "wv": np.ascontiguousarray(Wv[hs, :].T).astype(BF16),
            "wo": np.ascontiguousarray(Wo.T[:, hs]).astype(BF16),
            "wg": np.ascontiguousarray(Wg.T[:, hs]).astype(BF16),
            "bo": np.ascontiguousarray(bo[hs, None]),
            "bg": np.ascontiguousarray(bg[hs, None]),
        })
    return in_maps


def _run(inputs, trace=False):
    nc = _build()
    in_maps = _prep_in_maps(inputs)
    res = bass_utils.run_bass_kernel_spmd(
        nc, in_maps, core_ids=list(range(NCORES)), trace=trace,
    )
    full_T = np.concatenate([res.results[h]["outT"] for h in range(NCORES)], axis=0)
    out = np.ascontiguousarray(full_T.T).reshape(B, S, C_IN).astype(np.float32)
    return out, res


def kernel(**inputs):
    out, _ = _run(inputs, trace=False)
    return out


# revision 21
# speedup vs baseline: 2.2276x; 1.6601x over previous
"""Multi-head attention + sigmoid gating kernel for 8 TRN2 NeuronCores.

Problem (dense_transformer):
    C_IN=512, C=64, N_HEAD=8, B=4, S=2048
    q = (x @ Wq.T)/sqrt(C); k = x @ Wk.T; v = x @ Wv.T   (split heads)
    aw = q k^T + bias[h] + mask_offset;  P = softmax(aw)
    attn = P v ; out1 = attn @ Wo.T + bo
    out  = sigmoid(out1 @ Wg.T + bg) * out1

Sharding: one head per core for the attention part (bias[h] is the dominant
HBM tensor; each core reads exactly its own 16.8MB slice, reused across all
4 batches).  The output projection + gating is token-sharded: a per-sq-chunk
AllToAll re-shards attn.T from head-split to token-split, after which each
core computes complete out1/gating rows for its tokens with the full Wo/Wg.
No second collective is needed and the A2As overlap attention compute.

Orientation: features on partitions, tokens on free:
  - projections produce qT/kT (duplicated to 128 partitions for 2-way
    row-packed QK^T) and v65 = [v * mask01 | mask01] per 128-token tile.
    Zeroing masked keys' v-rows AND the sumexp-ones column is exactly
    equivalent to masking the softmax (both numerator and denominator).
  - QK^T produces aw.T (sk on partitions, sq free), two sk-tiles packed per
    PE pass into the two banks of a (128,1024) PSUM tile
  - P.T = exp(aw.T) * exp(bias[h]).T : one ACT pass per (128,1024) pair,
    one 2x-mode DVE tensor_mul per 512 half (exp(bias).T precomputed host-side)
  - PV: lhsT = v65 accumulates [attnU.T ; sumexp]; normalization via per-sqc
    batched reciprocal (bf16) + DRAM-broadcast + 2x DVE multiply
P2 is software-pipelined (PV pass lags one chunk) to keep PE dense/warm.
"""

import numpy as np
import ml_dtypes

import concourse.bass as bass
import concourse.mybir as mybir
import concourse.tile as tile
from concourse import bacc
from concourse import bass_utils

C_IN = 512
C = 64
H = 8
B = 4
S = 2048
TOK = B * S          # 8192
NCORES = 8
KT = C_IN // 128     # 4 contraction k-tiles
NCH = TOK // 512     # 16 token chunks of 512
SKT = S // 128       # 16 key tiles per batch
SQC = 4              # query chunks of 512 per batch
TC4 = 256            # tokens per core per sqc after A2A

BF16 = ml_dtypes.bfloat16
dtb = mybir.dt.bfloat16
dtf = mybir.dt.float32
AF = mybir.ActivationFunctionType
ALU = mybir.AluOpType


def _body(tc, xT, ebT, m01, wqk, wv, wo2, wg2, bo2, bg2, outT2):
    nc = tc.nc
    with (
        tc.tile_pool(name="const", bufs=1) as const,
        tc.tile_pool(name="resid", bufs=1) as resid,
        tc.tile_pool(name="xload", bufs=3) as xload,
        tc.tile_pool(name="agload", bufs=2) as agload,
        tc.tile_pool(name="ebload", bufs=4) as ebload,
        tc.tile_pool(name="ptp", bufs=22) as ptp,
        tc.tile_pool(name="pnp", bufs=3) as pnp,
        tc.tile_pool(name="anp", bufs=3) as anp,
        tc.tile_pool(name="o1p", bufs=2) as o1p,
        tc.tile_pool(name="small", bufs=2) as small,
        tc.tile_pool(name="psA", bufs=3, space="PSUM") as psA,
        tc.tile_pool(name="psB", bufs=2, space="PSUM") as psB,
        tc.tile_pool(name="dram", bufs=1, space="DRAM") as dram,
    ):
        # ---------------- constants ----------------
        w_qk = const.tile([128, KT, 128], dtb)
        nc.sync.dma_start(out=w_qk, in_=wqk.rearrange("(k p) m -> p k m", p=128))
        w_v = const.tile([128, KT, C], dtb)
        nc.sync.dma_start(out=w_v, in_=wv.rearrange("(k p) m -> p k m", p=128))
        w_o2 = const.tile([128, KT, C_IN], dtb)
        nc.sync.dma_start(out=w_o2, in_=wo2.rearrange("(k p) m -> p k m", p=128))
        w_g2 = const.tile([128, KT, C_IN], dtb)
        nc.sync.dma_start(out=w_g2, in_=wg2.rearrange("(k p) m -> p k m", p=128))
        m_01 = const.tile([128, B * SKT], dtf)
        nc.sync.dma_start(out=m_01, in_=m01)
        b_o2 = const.tile([128, KT], dtf)
        nc.sync.dma_start(out=b_o2, in_=bo2)
        b_g2 = const.tile([128, KT], dtf)
        nc.sync.dma_start(out=b_g2, in_=bg2)

        # ---------------- residents ----------------
        qTd = resid.tile([128, TOK], dtb)    # rows 0:64 and 64:128 both = q.T
        kTd = resid.tile([128, TOK], dtb)
        v65 = resid.tile([128, B * SKT, C + 1], dtb)  # [v*mask | mask] per token tile
        # ones column carries the key mask: masked keys drop out of sumexp
        nc.vector.tensor_copy(v65[:, :, C], m_01)

        # ---------------- P1: projections ----------------
        xTr = xT.rearrange("(k p) t -> p k t", p=128)
        for n in range(NCH):
            sl = slice(n * 512, (n + 1) * 512)
            xt = xload.tile([128, KT, 512], dtb, tag="xt")
            nc.sync.dma_start(out=xt, in_=xTr[:, :, sl])
            ps_qk = psA.tile([128, 512], dtf, tag="mm")
            for k in range(KT):
                nc.tensor.matmul(ps_qk, lhsT=w_qk[:, k, :], rhs=xt[:, k, :],
                                 start=(k == 0), stop=(k == KT - 1))
            nc.vector.tensor_copy(qTd[0:C, sl], ps_qk[0:C, :])
            nc.scalar.copy(qTd[C:128, sl], ps_qk[0:C, :])
            nc.vector.tensor_copy(kTd[0:C, sl], ps_qk[C:128, :])
            nc.scalar.copy(kTd[C:128, sl], ps_qk[C:128, :])
            for s4 in range(4):
                t = n * 4 + s4
                ps_v = psB.tile([128, C], dtf, tag="pv")
                for k in range(KT):
                    nc.tensor.matmul(ps_v, lhsT=xt[:, k, s4 * 128:(s4 + 1) * 128],
                                     rhs=w_v[:, k, :],
                                     start=(k == 0), stop=(k == KT - 1))
                # v rows of masked keys are zeroed (key-mask applied here)
                nc.scalar.mul(v65[:, t, 0:C], ps_v, m_01[:, t:t + 1])

        # ---------------- P2: attention (software pipelined) ----------------
        ebTr = ebT.rearrange("(j p) q -> p j q", p=128)
        a2aI = []
        a2aO = []
        for sqc in range(SQC):
            a2aI.append(dram.tile([C * NCORES, TC4], dtb, name=f"a2aI{sqc}"))
            a2aO.append(dram.tile([C * NCORES, TC4], dtb, name=f"a2aO{sqc}"))
        recd = dram.tile([SQC * B, 512], dtb)

        pend = None          # (sqc, b, pts) whose PV pass is not yet emitted
        attn_t = {}          # sqc -> sbuf tile (C, B*512)
        sexp_t = {}          # sqc -> sbuf tile (B, 512)

        def emit_pv(sqc, b, pts):
            ps_pv = psB.tile([C + 1, 512], dtf, tag="pv")
            for j in range(SKT):
                nc.tensor.matmul(ps_pv, lhsT=v65[:, b * SKT + j, :], rhs=pts[j],
                                 start=(j == 0), stop=(j == SKT - 1))
            nc.vector.tensor_copy(attn_t[sqc][:, b * 512:(b + 1) * 512], ps_pv[0:C, :])
            sx = small.tile([1, 512], dtf, tag="sx")
            nc.scalar.copy(sx, ps_pv[C:C + 1, :])
            nc.gpsimd.dma_start(out=sexp_t[sqc][b:b + 1, :], in_=sx)

        def emit_norm_and_a2a(sqc):
            rec = small.tile([B, 512], dtb, tag="rec")
            with nc.allow_low_precision(reason="1/sumexp in bf16 is within budget"):
                nc.vector.reciprocal(rec, sexp_t[sqc])
            nc.gpsimd.dma_start(out=recd[sqc * B:(sqc + 1) * B, :], in_=rec)
            for b in range(B):
                recB = small.tile([C, 512], dtb, tag="recB")
                row = recd[:][sqc * B + b:sqc * B + b + 1, :]
                nc.gpsimd.dma_start(out=recB, in_=bass.AP(
                    tensor=row.tensor, offset=row.offset,
                    ap=[[0, C]] + list(row.ap[1:])))
                at = attn_t[sqc][:, b * 512:(b + 1) * 512]
                nc.vector.tensor_mul(at, at, recB)
            # shard-major staging: row j*64+p, col i  =  attn_t[p, j*256+i]
            nc.gpsimd.dma_start(
                out=a2aI[sqc][:].rearrange("(j p) f -> p j f", p=C),
                in_=attn_t[sqc].rearrange("p (j f) -> p j f", j=NCORES))
            nc.gpsimd.collective_compute(
                "AllToAll", ALU.bypass,
                replica_groups=[list(range(NCORES))],
                ins=[a2aI[sqc].opt()], outs=[a2aO[sqc].opt()],
            )

        for sqc in range(SQC):
            ebt_lo = ebload.tile([128, SKT // 2, 512], dtb, tag="ebt")
            nc.sync.dma_start(out=ebt_lo,
                              in_=ebTr[:, 0:SKT // 2, sqc * 512:(sqc + 1) * 512])
            ebt_hi = ebload.tile([128, SKT // 2, 512], dtb, tag="ebt")
            nc.sync.dma_start(out=ebt_hi,
                              in_=ebTr[:, SKT // 2:SKT, sqc * 512:(sqc + 1) * 512])
            ebt_halves = (ebt_lo, ebt_hi)
            attn_t[sqc] = anp.tile([C, B * 512], dtb, tag="attn", name=f"attn_t{sqc}")
            sexp_t[sqc] = anp.tile([B, 512], dtf, tag="sexp", name=f"sexp_t{sqc}")
            for b in range(B):
                if pend is not None:
                    emit_pv(*pend)
                    if pend[1] == B - 1:
                        emit_norm_and_a2a(pend[0])
                    pend = None
                gq = b * S + sqc * 512
                pts = []
                for jp in range(SKT // 2):
                    jA, jB = 2 * jp, 2 * jp + 1
                    ps_aw = psA.tile([128, 1024], dtf, tag="mm")
                    nc.tensor.matmul(ps_aw[:, 0:512],
                                     lhsT=kTd[0:C, b * S + jA * 128:b * S + jA * 128 + 128],
                                     rhs=qTd[0:C, gq:gq + 512],
                                     start=True, stop=True)
                    nc.tensor.matmul(ps_aw[:, 512:1024],
                                     lhsT=kTd[C:128, b * S + jB * 128:b * S + jB * 128 + 128],
                                     rhs=qTd[C:128, gq:gq + 512],
                                     start=True, stop=True)
                    pn = pnp.tile([128, 1024], dtb, tag="pn")
                    nc.scalar.activation(pn, ps_aw, AF.Exp)
                    for half, j in ((0, jA), (1, jB)):
                        pt = ptp.tile([128, 512], dtb, tag="pt")
                        nc.vector.tensor_mul(
                            pt, pn[:, half * 512:(half + 1) * 512],
                            ebt_halves[j // (SKT // 2)][:, j % (SKT // 2), :])
                        pts.append(pt)
                pend = (sqc, b, pts)
        emit_pv(*pend)
        emit_norm_and_a2a(pend[0])

        # ---------------- P4+P6: token-sharded out-proj + gating ----------------
        for sqc in range(SQC):
            atg = agload.tile([128, KT, TC4], dtb, tag="atg")
            nc.sync.dma_start(out=atg,
                              in_=a2aO[sqc][:].rearrange("(k p) f -> p k f", p=128))
            out1b = o1p.tile([128, KT, TC4], dtb, tag="out1", name=f"out1b{sqc}")
            for fo in range(KT):
                ps_o1 = psA.tile([128, TC4], dtf, tag="mm")
                for k in range(KT):
                    nc.tensor.matmul(ps_o1, lhsT=w_o2[:, k, fo * 128:(fo + 1) * 128],
                                     rhs=atg[:, k, :],
                                     start=(k == 0), stop=(k == KT - 1))
                nc.scalar.add(out1b[:, fo, :], ps_o1, b_o2[:, fo:fo + 1])
            for go in range(KT):
                ps_o2 = psA.tile([128, TC4], dtf, tag="mm")
                for k in range(KT):
                    nc.tensor.matmul(ps_o2, lhsT=w_g2[:, k, go * 128:(go + 1) * 128],
                                     rhs=out1b[:, k, :],
                                     start=(k == 0), stop=(k == KT - 1))
                sig = small.tile([128, TC4], dtf, tag="sig")
                nc.scalar.activation(sig, ps_o2, AF.Sigmoid, bias=b_g2[:, go:go + 1],
                                     scale=1.0)
                fin = small.tile([128, TC4], dtf, tag="fin")
                nc.vector.tensor_mul(fin, sig, out1b[:, go, :])
                nc.sync.dma_start(
                    out=outT2[go * 128:(go + 1) * 128, sqc * TC4:(sqc + 1) * TC4],
                    in_=fin)


_CACHE = {}


def _build():
    if "nc" in _CACHE:
        return _CACHE["nc"]
    nc = bacc.Bacc("TRN2", target_bir_lowering=False, debug=False, num_devices=NCORES)
    aps = {}
    aps["xT"] = nc.dram_tensor("xT", [C_IN, TOK], dtb, kind="ExternalInput").ap()
    aps["ebT"] = nc.dram_tensor("ebT", [S, S], dtb, kind="ExternalInput").ap()
    aps["m01"] = nc.dram_tensor("m01", [128, B * SKT], dtf, kind="ExternalInput").ap()
    aps["wqk"] = nc.dram_tensor("wqk", [C_IN, 128], dtb, kind="ExternalInput").ap()
    aps["wv"] = nc.dram_tensor("wv", [C_IN, C], dtb, kind="ExternalInput").ap()
    aps["wo2"] = nc.dram_tensor("wo2", [C_IN, C_IN], dtb, kind="ExternalInput").ap()
    aps["wg2"] = nc.dram_tensor("wg2", [C_IN, C_IN], dtb, kind="ExternalInput").ap()
    aps["bo2"] = nc.dram_tensor("bo2", [128, KT], dtf, kind="ExternalInput").ap()
    aps["bg2"] = nc.dram_tensor("bg2", [128, KT], dtf, kind="ExternalInput").ap()
    aps["outT2"] = nc.dram_tensor("outT2", [C_IN, SQC * TC4], dtf,
                                  kind="ExternalOutput").ap()
    with tile.TileContext(nc) as tc:
        _body(tc, **aps)
    nc.compile()
    _CACHE["nc"] = nc
    return nc


def _prep_in_maps(inputs):
    x = np.asarray(inputs["x"], dtype=np.float32)
    bias = np.asarray(inputs["bias"], dtype=np.float32)
    mask = np.asarray(inputs["attention_mask"])
    Wq = np.asarray(inputs["Wq"], dtype=np.float32)
    Wk = np.asarray(inputs["Wk"], dtype=np.float32)
    Wv = np.asarray(inputs["Wv"], dtype=np.float32)
    Wo = np.asarray(inputs["Wo"], dtype=np.float32)
    bo = np.asarray(inputs["bo"], dtype=np.float32)
    Wg = np.asarray(inputs["Wg"], dtype=np.float32)
    bg = np.asarray(inputs["bg"], dtype=np.float32)

    xT = np.ascontiguousarray(x.reshape(TOK, C_IN).T).astype(BF16)
    # mask columns: m01[p, b*SKT + j] = 0.0 where mask[b, j*128+p]==0 else 1.0
    m01 = (mask.reshape(B, SKT, 128) != 0).astype(np.float32)
    m01 = np.ascontiguousarray(m01.transpose(2, 0, 1).reshape(128, B * SKT))
    wo2 = np.ascontiguousarray(Wo.T).astype(BF16)
    wg2 = np.ascontiguousarray(Wg.T).astype(BF16)
    bo2 = np.ascontiguousarray(bo.reshape(KT, 128).T)
    bg2 = np.ascontiguousarray(bg.reshape(KT, 128).T)

    in_maps = []
    for h in range(NCORES):
        hs = slice(h * C, (h + 1) * C)
        ebT = np.ascontiguousarray(np.exp(bias[h]).T).astype(BF16)
        wqk = np.empty((C_IN, 128), dtype=BF16)
        wqk[:, 0:C] = (Wq[hs, :] / np.sqrt(C)).T.astype(BF16)
        wqk[:, C:128] = Wk[hs, :].T.astype(BF16)
        in_maps.append({
            "xT": xT,
            "ebT": ebT,
            "m01": m01,
            "wqk": wqk,
            "wv": np.ascontiguousarray(Wv[hs, :].T).astype(BF16),
            "wo2": wo2,
            "wg2": wg2,
            "bo2": bo2,
            "bg2": bg2,
        })
    return in_maps


def _run(inputs, trace=False):
    nc = _build()
    in_maps = _prep_in_maps(inputs)
    res = bass_utils.run_bass_kernel_spmd(
        nc, in_maps, core_ids=list(range(NCORES)), trace=trace,
    )
    # un-permute: core c, sqc, i  ->  token (b = (c*256+i)//512,
    #                                        s = sqc*512 + (c*256+i)%512)
    out_full = np.empty((TOK, C_IN), dtype=np.float32)
    idx = np.arange(TC4)
    for c in range(NCORES):
        o = res.results[c]["outT2"]
        cc = c * TC4 + idx
        b = cc // 512
        si = cc % 512
        for sqc in range(SQC):
            out_full[b * S + sqc * 512 + si, :] = o[:, sqc * TC4:(sqc + 1) * TC4].T
    out = out_full.reshape(B, S, C_IN)
    return out, res


def kernel(**inputs):
    out, _ = _run(inputs, trace=False)
    return out
